# revision 4
# baseline (speedup 1.0000x reference)
"""Causal single-head attention (B=4, S=4096, D=1024, fp32) on 8 TRN2 cores.

Sharding: 8 cores = 4 batches x 2 roles (one SPMD NEFF, role picked by
partition_id):
  role A (cores 0-3, batch = pid):     queries [0, SPLIT),  kv [0, SPLIT)
  role B (cores 4-7, batch = pid - 4): queries [SPLIT, S),  kv [0, S)
SPLIT=2944 balances causal-attention + projection FLOPs across roles.

Per-core pipeline (bf16 matmuls, fp32 PSUM accumulation):
  1. Project kT/v (then qT) from streamed xT tiles.
  2. Per query block: scores computed transposed (S^T[kv, q]) so the exp
     output P^T feeds the PV matmul directly; causal handled by only
     iterating kv chunks up to the block end plus additive -1e9 masks on
     diagonal chunks; softmax denominator via a ones-vector matmul;
     normalization via K=1 broadcast matmul of the reciprocal fused into
     the PSUM->SBUF copy.
Output per core is O^T [D, S] (its query-column range); host transposes.
"""

import numpy as np
import ml_dtypes

BF16 = ml_dtypes.bfloat16

B, S, D = 4, 4096, 1024
SPLIT = 2944
N_CORES = 8
SCALE = 1.0 / np.sqrt(np.float32(D))  # 1/32
NEG = -1.0e9

_PROGRAM = None


def _role_blocks(q0, q1, m_block):
    """List of (m_start, m_width) query blocks covering [q0, q1)."""
    blocks = []
    m = q0
    while m < q1:
        blocks.append((m, min(m_block, q1 - m)))
        m += m_block
    return blocks


def _build_role(tc, nc, aps, q0, q1, kv_len, m_block, tag, d=D):
    import concourse.bass as bass  # noqa: F401
    from concourse import mybir
    from contextlib import ExitStack

    f32 = mybir.dt.float32
    bf16 = mybir.dt.bfloat16
    Exp = mybir.ActivationFunctionType.Exp
    add_op = mybir.AluOpType.add
    scale = float(1.0 / np.sqrt(np.float32(d)))

    xT, wqT, wkT, wvT, masks, oT = (
        aps["xT"], aps["wqT"], aps["wkT"], aps["wvT"], aps["masks"], aps["oT"],
    )

    DCH = d // 128            # d-chunks
    q_len = q1 - q0
    n_kv = kv_len // 128      # kv chunks
    blocks = _role_blocks(q0, q1, m_block)

    with ExitStack() as ctx:
        # ---- persistent SBUF: kT, v, qT, masks, ones -------------------
        kt_pool = ctx.enter_context(tc.tile_pool(name=f"kt{tag}", bufs=DCH))
        qt_pool = ctx.enter_context(tc.tile_pool(name=f"qt{tag}", bufs=DCH))
        v_pool = ctx.enter_context(tc.tile_pool(name=f"v{tag}", bufs=n_kv))
        misc_pool = ctx.enter_context(tc.tile_pool(name=f"misc{tag}", bufs=1))

        kt = [kt_pool.tile([128, kv_len], bf16, tag="kt", name=f"kt{i}") for i in range(DCH)]
        qt = [qt_pool.tile([128, q_len], bf16, tag="qt", name=f"qt{i}") for i in range(DCH)]
        v = [v_pool.tile([128, d], bf16, tag="v", name=f"v{i}") for i in range(n_kv)]

        masks_sb = misc_pool.tile([128, 4, 512], f32, tag="masks")
        nc.sync.dma_start(
            masks_sb[:], masks.rearrange("(a p) m -> p a m", p=128)
        )
        ones_col = misc_pool.tile([128, 1], bf16, tag="ones_col")
        nc.gpsimd.memset(ones_col[:], 1.0)
        ones_row = misc_pool.tile([1, 128], f32, tag="ones_row")
        nc.gpsimd.memset(ones_row[:], 1.0)

        # ---- phase 1a: kT and v projections ----------------------------
        def s_tiles(lo, hi):
            t = lo
            while t < hi:
                yield (t, min(512, hi - t))
                t += 512

        with tc.tile_pool(name=f"wkv{tag}", bufs=1) as w_pool, \
             tc.tile_pool(name=f"xt{tag}", bufs=2) as xt_pool, \
             tc.tile_pool(name=f"pps{tag}", bufs=4, space="PSUM") as proj_ps:
            wk_sb = w_pool.tile([128, DCH, d], bf16, tag="wk")
            wv_sb = w_pool.tile([128, DCH, d], bf16, tag="wv")
            nc.sync.dma_start(wk_sb[:], wkT.rearrange("(a p) m -> p a m", p=128))
            nc.sync.dma_start(wv_sb[:], wvT.rearrange("(a p) m -> p a m", p=128))

            for s0, sw in s_tiles(0, kv_len):
                xt = xt_pool.tile([128, DCH, 512], bf16, tag="xt")
                nc.sync.dma_start(
                    xt[:, :, :sw],
                    xT[:, s0:s0 + sw].rearrange("(a p) m -> p a m", p=128),
                )
                # kT[d_out chunk i, s] = sum_j WkT[j, i].T @ xT[j, s]
                for i in range(DCH):
                    ps = proj_ps.tile([128, 512], f32, tag="pps")
                    for j in range(DCH):
                        nc.tensor.matmul(
                            ps[:, :sw],
                            wk_sb[:, j, i * 128:(i + 1) * 128],
                            xt[:, j, :sw],
                            start=(j == 0), stop=(j == DCH - 1),
                        )
                    nc.scalar.copy(kt[i][:, s0:s0 + sw], ps[:, :sw])
                # v[s chunk c, d_out] = sum_j (xT[j, c]).T @ WvT[j, :]
                for c in range(sw // 128):
                    for h0 in range(0, d, 512):
                        hw_ = min(512, d - h0)
                        ps = proj_ps.tile([128, 512], f32, tag="pps")
                        for j in range(DCH):
                            nc.tensor.matmul(
                                ps[:, :hw_],
                                xt[:, j, c * 128:(c + 1) * 128],
                                wv_sb[:, j, h0:h0 + hw_],
                                start=(j == 0), stop=(j == DCH - 1),
                            )
                        nc.scalar.copy(
                            v[s0 // 128 + c][:, h0:h0 + hw_], ps[:, :hw_]
                        )

        # ---- phase 1b: qT projection -----------------------------------
        with tc.tile_pool(name=f"wq{tag}", bufs=1) as w_pool, \
             tc.tile_pool(name=f"xtq{tag}", bufs=2) as xt_pool, \
             tc.tile_pool(name=f"qps{tag}", bufs=4, space="PSUM") as proj_ps:
            wq_sb = w_pool.tile([128, DCH, d], bf16, tag="wq")
            nc.sync.dma_start(wq_sb[:], wqT.rearrange("(a p) m -> p a m", p=128))
            for s0, sw in s_tiles(q0, q1):
                xt = xt_pool.tile([128, DCH, 512], bf16, tag="xtq")
                nc.sync.dma_start(
                    xt[:, :, :sw],
                    xT[:, s0:s0 + sw].rearrange("(a p) m -> p a m", p=128),
                )
                for i in range(DCH):
                    ps = proj_ps.tile([128, 512], f32, tag="qps")
                    for j in range(DCH):
                        nc.tensor.matmul(
                            ps[:, :sw],
                            wq_sb[:, j, i * 128:(i + 1) * 128],
                            xt[:, j, :sw],
                            start=(j == 0), stop=(j == DCH - 1),
                        )
                    nc.scalar.copy(qt[i][:, s0 - q0:s0 - q0 + sw], ps[:, :sw])

        # ---- phase 2: attention per query block ------------------------
        n_chunks_max = (blocks[-1][0] + blocks[-1][1]) // 128
        with tc.tile_pool(name=f"pt{tag}", bufs=n_chunks_max + 1) as pt_pool, \
             tc.tile_pool(name=f"att{tag}", bufs=2) as att_sb, \
             tc.tile_pool(name=f"ob{tag}", bufs=3) as out_sb, \
             tc.tile_pool(name=f"st{tag}", bufs=2, space="PSUM") as st_ps, \
             tc.tile_pool(name=f"dn{tag}", bufs=2, space="PSUM") as dn_ps, \
             tc.tile_pool(name=f"ot{tag}", bufs=2, space="PSUM") as ot_ps, \
             tc.tile_pool(name=f"bc{tag}", bufs=1, space="PSUM") as bc_ps:
            for m0, mw in blocks:
                mloc = m0 - q0
                n_chunks = (m0 + mw) // 128
                denom = dn_ps.tile([1, m_block], f32, tag="dn")
                pts = []
                for n in range(n_chunks):
                    st = st_ps.tile([128, m_block], f32, tag="st")
                    for j in range(DCH):
                        nc.tensor.matmul(
                            st[:, :mw],
                            kt[j][:, n * 128:(n + 1) * 128],
                            qt[j][:, mloc:mloc + mw],
                            start=(j == 0), stop=(j == DCH - 1),
                        )
                    rel = n * 128 - m0
                    if rel >= 0:
                        nc.vector.tensor_tensor(
                            st[:, :mw], st[:, :mw],
                            masks_sb[:, rel // 128, :mw], add_op,
                        )
                    pt = pt_pool.tile([128, m_block], bf16, tag="pt", name="pt")
                    nc.scalar.activation(pt[:, :mw], st[:, :mw], Exp, scale=scale)
                    pts.append(pt)
                    nc.tensor.matmul(
                        denom[:, :mw], ones_col[:], pt[:, :mw],
                        start=(n == 0), stop=(n == n_chunks - 1),
                    )
                recip = att_sb.tile([1, m_block], f32, tag="recip")
                nc.vector.reciprocal(recip[:, :mw], denom[:, :mw])
                bcast_ps = bc_ps.tile([128, m_block], f32, tag="bc")
                nc.tensor.matmul(
                    bcast_ps[:, :mw], ones_row[:], recip[:, :mw],
                    start=True, stop=True,
                )
                bcast = att_sb.tile([128, m_block], f32, tag="bcast")
                nc.scalar.copy(bcast[:, :mw], bcast_ps[:, :mw])
                for dd in range(DCH):
                    ot = ot_ps.tile([128, m_block], f32, tag="ot")
                    for n in range(n_chunks):
                        nc.tensor.matmul(
                            ot[:, :mw],
                            v[n][:, dd * 128:(dd + 1) * 128],
                            pts[n][:, :mw],
                            start=(n == 0), stop=(n == n_chunks - 1),
                        )
                    o = out_sb.tile([128, m_block], f32, tag="o")
                    nc.vector.tensor_mul(o[:, :mw], ot[:, :mw], bcast[:, :mw])
                    nc.sync.dma_start(
                        oT[dd * 128:(dd + 1) * 128, m0:m0 + mw], o[:, :mw]
                    )


def build_program(s=S, d=D, split=SPLIT, m_block_a=512, m_block_b=384,
                  n_cores=N_CORES):
    """Build and compile the SPMD Bass program. Returns the Bacc object."""
    import concourse.tile as tile
    from concourse import bacc, mybir

    nc = bacc.Bacc(
        "TRN2",
        target_bir_lowering=False,
        debug=False,
        enable_asserts=False,
        num_devices=n_cores,
    )
    bf16 = mybir.dt.bfloat16
    f32 = mybir.dt.float32
    aps = {
        "xT": nc.dram_tensor("xT", [d, s], bf16, kind="ExternalInput").ap(),
        "wqT": nc.dram_tensor("wqT", [d, d], bf16, kind="ExternalInput").ap(),
        "wkT": nc.dram_tensor("wkT", [d, d], bf16, kind="ExternalInput").ap(),
        "wvT": nc.dram_tensor("wvT", [d, d], bf16, kind="ExternalInput").ap(),
        "masks": nc.dram_tensor("masks", [512, 512], f32, kind="ExternalInput").ap(),
        "oT": nc.dram_tensor("oT", [d, s], f32, kind="ExternalOutput").ap(),
    }
    with tile.TileContext(nc) as tc:
        pid = nc.partition_id()
        with tc.If(pid < n_cores // 2) as cmp:
            _build_role(tc, nc, aps, 0, split, split, m_block_a, "a", d=d)
        with cmp.Else():
            _build_role(tc, nc, aps, split, s, s, m_block_b, "b", d=d)
    nc.compile()
    return nc


def host_masks():
    part = np.arange(128, dtype=np.int64)[:, None]
    col = np.arange(512, dtype=np.int64)[None, :]
    m = np.zeros((4, 128, 512), np.float32)
    for r in range(4):
        m[r] = np.where(col >= part + r * 128, 0.0, NEG)
    return np.ascontiguousarray(m.reshape(512, 512))


def make_in_maps(x, Wq, Wk, Wv):
    wqT = np.ascontiguousarray(Wq.T.astype(BF16))
    wkT = np.ascontiguousarray(Wk.T.astype(BF16))
    wvT = np.ascontiguousarray(Wv.T.astype(BF16))
    masks = host_masks()
    xT = np.ascontiguousarray(x.astype(BF16).transpose(0, 2, 1))  # [B, D, S]
    in_maps = []
    for c in range(N_CORES):
        b = c % B
        in_maps.append({
            "xT": xT[b], "wqT": wqT, "wkT": wkT, "wvT": wvT, "masks": masks,
        })
    return in_maps


def gather_output(results):
    out = np.empty((B, S, D), np.float32)
    for b in range(B):
        oA = results[b]["oT"]        # [D, S], valid cols [0, SPLIT)
        oB = results[B + b]["oT"]    # [D, S], valid cols [SPLIT, S)
        out[b, :SPLIT] = oA[:, :SPLIT].T
        out[b, SPLIT:] = oB[:, SPLIT:].T
    return out


def get_program():
    global _PROGRAM
    if _PROGRAM is None:
        _PROGRAM = build_program()
    return _PROGRAM


def kernel(x, Wq, Wk, Wv, _trace=False, _trace_cores=None):
    from concourse import bass_utils

    nc = get_program()
    in_maps = make_in_maps(x, Wq, Wk, Wv)
    res = bass_utils.run_bass_kernel_spmd(
        nc, in_maps, core_ids=list(range(N_CORES)),
        trace=_trace, trace_cores=_trace_cores,
    )
    out = gather_output(res.results)
    if _trace:
        kernel.last_results = res
    return out


# revision 10
# speedup vs baseline: 1.0798x; 1.0798x over previous
"""Causal single-head attention (B=4, S=4096, D=1024, fp32) on 8 TRN2 cores.

Sharding: 8 cores = 4 batches x 2 roles (one SPMD NEFF, role picked by
partition_id):
  role A (cores 0-3, batch = pid):     queries [0, SPLIT),  kv [0, SPLIT)
  role B (cores 4-7, batch = pid - 4): queries [SPLIT, S),  kv [0, S)
SPLIT=2944 balances causal-attention + projection FLOPs across roles.

Per-core pipeline (bf16 matmuls, fp32 PSUM accumulation):
  1. Project kT/v (then qT) from streamed xT tiles.
  2. Per query block: scores computed transposed (S^T[kv, q]) so the exp
     output P^T feeds the PV matmul directly; causal handled by only
     iterating kv chunks up to the block end plus additive -1e9 masks on
     diagonal chunks; softmax denominator via a ones-vector matmul;
     normalization via K=1 broadcast matmul of the reciprocal fused into
     the PSUM->SBUF copy.
Output per core is O^T [D, S] (its query-column range); host transposes.
"""

import numpy as np
import ml_dtypes

BF16 = ml_dtypes.bfloat16

B, S, D = 4, 4096, 1024
SPLIT = 2944
N_CORES = 8
SCALE = 1.0 / np.sqrt(np.float32(D))  # 1/32
NEG = -1.0e9

_PROGRAM = None


def _role_blocks(q0, q1, m_block):
    """List of (m_start, m_width) query blocks covering [q0, q1)."""
    blocks = []
    m = q0
    while m < q1:
        blocks.append((m, min(m_block, q1 - m)))
        m += m_block
    return blocks


def _build_role(tc, nc, aps, q0, q1, kv_len, m_block, tag, d=D):
    import concourse.bass as bass  # noqa: F401
    from concourse import mybir
    from contextlib import ExitStack

    f32 = mybir.dt.float32
    bf16 = mybir.dt.bfloat16
    Exp = mybir.ActivationFunctionType.Exp
    add_op = mybir.AluOpType.add
    scale = float(1.0 / np.sqrt(np.float32(d)))

    xT, wqT, wkT, wvT, masks, oT = (
        aps["xT"], aps["wqT"], aps["wkT"], aps["wvT"], aps["masks"], aps["oT"],
    )

    DCH = d // 128            # d-chunks
    q_len = q1 - q0
    n_kv = kv_len // 128      # kv chunks
    blocks = _role_blocks(q0, q1, m_block)

    with ExitStack() as ctx:
        # ---- persistent SBUF: kT, v, qT, masks, ones -------------------
        kt_pool = ctx.enter_context(tc.tile_pool(name=f"kt{tag}", bufs=DCH))
        qt_pool = ctx.enter_context(tc.tile_pool(name=f"qt{tag}", bufs=DCH))
        v_pool = ctx.enter_context(tc.tile_pool(name=f"v{tag}", bufs=n_kv))
        misc_pool = ctx.enter_context(tc.tile_pool(name=f"misc{tag}", bufs=1))

        kt = [kt_pool.tile([128, kv_len], bf16, tag="kt", name=f"kt{i}") for i in range(DCH)]
        qt = [qt_pool.tile([128, q_len], bf16, tag="qt", name=f"qt{i}") for i in range(DCH)]
        v = [v_pool.tile([128, d], bf16, tag="v", name=f"v{i}") for i in range(n_kv)]

        masks_sb = misc_pool.tile([128, 4, 512], bf16, tag="masks")
        nc.sync.dma_start(
            masks_sb[:], masks.rearrange("(a p) m -> p a m", p=128)
        )
        ones_row = misc_pool.tile([1, 128], f32, tag="ones_row")
        nc.gpsimd.memset(ones_row[:], 1.0)
        ones_col = misc_pool.tile([128, 1], f32, tag="ones_col")
        nc.gpsimd.memset(ones_col[:], 1.0)

        # ---- phase 1a: kT and v projections ----------------------------
        def s_tiles(lo, hi):
            t = lo
            while t < hi:
                yield (t, min(512, hi - t))
                t += 512

        with tc.tile_pool(name=f"xt{tag}", bufs=12) as xt_pool, \
             tc.tile_pool(name=f"pps{tag}", bufs=4, space="PSUM") as proj_ps:

            def load_xt(s0, sw):
                xts = []
                for j in range(DCH):
                    t = xt_pool.tile([128, 512], bf16, tag="xt", name=f"xt{j}")
                    nc.sync.dma_start(
                        t[:, :sw], xT[j * 128:(j + 1) * 128, s0:s0 + sw]
                    )
                    xts.append(t)
                return xts

            def proj_pass(w_sb, lo, hi, out_cb):
                # out_cb(i, s0, sw, psum_slice) consumes the [128, sw]
                # projection of d_out chunk i for tokens [s0, s0+sw)
                for s0, sw in s_tiles(lo, hi):
                    xts = load_xt(s0, sw)
                    for i in range(DCH):
                        ps = proj_ps.tile([128, 512], f32, tag="pps")
                        for j in range(DCH):
                            nc.tensor.matmul(
                                ps[:, :sw],
                                w_sb[:, j, i * 128:(i + 1) * 128],
                                xts[j][:, :sw],
                                start=(j == 0), stop=(j == DCH - 1),
                            )
                        out_cb(i, s0, sw, ps)
                    yield s0, sw, xts

            # per-chunk weight DMAs so the first matmuls start as soon as
            # chunk 0 lands instead of waiting for the whole 2MB transfer
            with tc.tile_pool(name=f"wkv{tag}", bufs=1) as w_pool:
                wk_sb = w_pool.tile([128, DCH, d], bf16, tag="wk")
                wv_sb = w_pool.tile([128, DCH, d], bf16, tag="wv")
                for j in range(DCH):
                    nc.sync.dma_start(wk_sb[:, j, :], wkT[j * 128:(j + 1) * 128, :])
                for j in range(DCH):
                    nc.sync.dma_start(wv_sb[:, j, :], wvT[j * 128:(j + 1) * 128, :])

                def kt_cb(i, s0, sw, ps):
                    nc.scalar.copy(kt[i][:, s0:s0 + sw], ps[:, :sw])

                for s0, sw, xts in proj_pass(wk_sb, 0, kv_len, kt_cb):
                    # v[s chunk c, d_out] = sum_j (xT[j, c]).T @ WvT[j, :]
                    for c in range(sw // 128):
                        for h0 in range(0, d, 512):
                            hw_ = min(512, d - h0)
                            ps = proj_ps.tile([128, 512], f32, tag="pps")
                            for j in range(DCH):
                                nc.tensor.matmul(
                                    ps[:, :hw_],
                                    xts[j][:, c * 128:(c + 1) * 128],
                                    wv_sb[:, j, h0:h0 + hw_],
                                    start=(j == 0), stop=(j == DCH - 1),
                                )
                            nc.scalar.copy(
                                v[s0 // 128 + c][:, h0:h0 + hw_], ps[:, :hw_]
                            )

            # qT projection (wq pool reuses the freed wk/wv space; per-chunk
            # DMAs keep the WAR stall at the transition ~1 chunk deep)
            with tc.tile_pool(name=f"wq{tag}", bufs=1) as w_pool:
                wq_sb = w_pool.tile([128, DCH, d], bf16, tag="wq")
                for j in range(DCH):
                    nc.sync.dma_start(wq_sb[:, j, :], wqT[j * 128:(j + 1) * 128, :])

                def qt_cb(i, s0, sw, ps):
                    nc.scalar.copy(qt[i][:, s0 - q0:s0 - q0 + sw], ps[:, :sw])

                for _ in proj_pass(wq_sb, q0, q1, qt_cb):
                    pass

        # ---- phase 2: attention per query block ------------------------
        # Diagonal chunks are clipped to their valid column range [lo, mw):
        # for a chunk starting at kv position n0 = m0 + rel (rel >= 0),
        # columns [0, rel) of the block are entirely masked, so QK/exp/PV
        # skip them.
        n_chunks_max = (blocks[-1][0] + blocks[-1][1]) // 128
        with tc.tile_pool(name=f"pt{tag}", bufs=n_chunks_max + 1) as pt_pool, \
             tc.tile_pool(name=f"att{tag}", bufs=2) as att_sb, \
             tc.tile_pool(name=f"ob{tag}", bufs=3) as out_sb, \
             tc.tile_pool(name=f"st{tag}", bufs=2, space="PSUM") as st_ps, \
             tc.tile_pool(name=f"ot{tag}", bufs=2, space="PSUM") as ot_ps, \
             tc.tile_pool(name=f"bc{tag}", bufs=1, space="PSUM") as bc_ps:
            for m0, mw in blocks:
                mloc = m0 - q0
                n_chunks = (m0 + mw) // 128
                acc = att_sb.tile([128, m_block], f32, tag="acc", name="acc")
                pts = []
                for n in range(n_chunks):
                    rel = n * 128 - m0
                    lo = max(rel, 0)
                    st = st_ps.tile([128, m_block], f32, tag="st")
                    for j in range(DCH):
                        nc.tensor.matmul(
                            st[:, lo:mw],
                            kt[j][:, n * 128:(n + 1) * 128],
                            qt[j][:, mloc + lo:mloc + mw],
                            start=(j == 0), stop=(j == DCH - 1),
                        )
                    if rel >= 0:
                        nc.vector.tensor_tensor(
                            st[:, lo:mw], st[:, lo:mw],
                            masks_sb[:, rel // 128, lo:mw], add_op,
                        )
                    pt = pt_pool.tile([128, m_block], bf16, tag="pt", name="pt")
                    nc.scalar.activation(pt[:, lo:mw], st[:, lo:mw], Exp, scale=scale)
                    pts.append(pt)
                    # accumulate exp tiles (fp32) for the softmax denominator
                    if n == 0:
                        nc.vector.tensor_copy(acc[:, :mw], pt[:, :mw])
                    else:
                        nc.vector.tensor_add(acc[:, lo:mw], acc[:, lo:mw],
                                             pt[:, lo:mw])
                # denominator = partition-sum of acc via one fp32 ones-matmul
                dn_ps = bc_ps.tile([1, m_block], f32, tag="dnp", name="dn_ps")
                nc.tensor.matmul(
                    dn_ps[:, :mw], ones_col[:], acc[:, :mw],
                    start=True, stop=True,
                )
                recip = att_sb.tile([1, m_block], f32, tag="recip")
                nc.vector.reciprocal(recip[:, :mw], dn_ps[:, :mw])
                bcast_ps = bc_ps.tile([128, m_block], f32, tag="bc")
                nc.tensor.matmul(
                    bcast_ps[:, :mw], ones_row[:], recip[:, :mw],
                    start=True, stop=True,
                )
                bcast = att_sb.tile([128, m_block], f32, tag="bcast")
                nc.scalar.copy(bcast[:, :mw], bcast_ps[:, :mw])
                for dd in range(DCH):
                    ot = ot_ps.tile([128, m_block], f32, tag="ot")
                    for n in range(n_chunks):
                        lo = max(n * 128 - m0, 0)
                        nc.tensor.matmul(
                            ot[:, lo:mw],
                            v[n][:, dd * 128:(dd + 1) * 128],
                            pts[n][:, lo:mw],
                            start=(n == 0), stop=(n == n_chunks - 1),
                        )
                    o = out_sb.tile([128, m_block], f32, tag="o")
                    nc.vector.tensor_mul(o[:, :mw], ot[:, :mw], bcast[:, :mw])
                    nc.sync.dma_start(
                        oT[dd * 128:(dd + 1) * 128, m0:m0 + mw], o[:, :mw]
                    )


def build_program(s=S, d=D, split=SPLIT, m_block_a=512, m_block_b=384,
                  n_cores=N_CORES):
    """Build and compile the SPMD Bass program. Returns the Bacc object."""
    import concourse.tile as tile
    from concourse import bacc, mybir

    nc = bacc.Bacc(
        "TRN2",
        target_bir_lowering=False,
        debug=False,
        enable_asserts=False,
        num_devices=n_cores,
    )
    bf16 = mybir.dt.bfloat16
    f32 = mybir.dt.float32
    aps = {
        "xT": nc.dram_tensor("xT", [d, s], bf16, kind="ExternalInput").ap(),
        "wqT": nc.dram_tensor("wqT", [d, d], bf16, kind="ExternalInput").ap(),
        "wkT": nc.dram_tensor("wkT", [d, d], bf16, kind="ExternalInput").ap(),
        "wvT": nc.dram_tensor("wvT", [d, d], bf16, kind="ExternalInput").ap(),
        "masks": nc.dram_tensor("masks", [512, 512], bf16, kind="ExternalInput").ap(),
        "oT": nc.dram_tensor("oT", [d, s], f32, kind="ExternalOutput").ap(),
    }
    with tile.TileContext(nc) as tc:
        pid = nc.partition_id()
        with tc.If(pid < n_cores // 2) as cmp:
            _build_role(tc, nc, aps, 0, split, split, m_block_a, "a", d=d)
        with cmp.Else():
            _build_role(tc, nc, aps, split, s, s, m_block_b, "b", d=d)
    nc.compile()
    return nc


def host_masks():
    part = np.arange(128, dtype=np.int64)[:, None]
    col = np.arange(512, dtype=np.int64)[None, :]
    m = np.zeros((4, 128, 512), np.float32)
    for r in range(4):
        m[r] = np.where(col >= part + r * 128, 0.0, NEG)
    return np.ascontiguousarray(m.reshape(512, 512).astype(BF16))


def make_in_maps(x, Wq, Wk, Wv):
    wqT = np.ascontiguousarray(Wq.T.astype(BF16))
    wkT = np.ascontiguousarray(Wk.T.astype(BF16))
    wvT = np.ascontiguousarray(Wv.T.astype(BF16))
    masks = host_masks()
    xT = np.ascontiguousarray(x.astype(BF16).transpose(0, 2, 1))  # [B, D, S]
    in_maps = []
    for c in range(N_CORES):
        b = c % B
        in_maps.append({
            "xT": xT[b], "wqT": wqT, "wkT": wkT, "wvT": wvT, "masks": masks,
        })
    return in_maps


def gather_output(results):
    out = np.empty((B, S, D), np.float32)
    for b in range(B):
        oA = results[b]["oT"]        # [D, S], valid cols [0, SPLIT)
        oB = results[B + b]["oT"]    # [D, S], valid cols [SPLIT, S)
        out[b, :SPLIT] = oA[:, :SPLIT].T
        out[b, SPLIT:] = oB[:, SPLIT:].T
    return out


def get_program():
    global _PROGRAM
    if _PROGRAM is None:
        _PROGRAM = build_program()
    return _PROGRAM


def kernel(x, Wq, Wk, Wv, _trace=False, _trace_cores=None):
    from concourse import bass_utils

    nc = get_program()
    in_maps = make_in_maps(x, Wq, Wk, Wv)
    res = bass_utils.run_bass_kernel_spmd(
        nc, in_maps, core_ids=list(range(N_CORES)),
        trace=_trace, trace_cores=_trace_cores,
    )
    out = gather_output(res.results)
    if _trace:
        kernel.last_results = res
    return out


# revision 11
# speedup vs baseline: 1.0953x; 1.0144x over previous
"""Causal single-head attention (B=4, S=4096, D=1024, fp32) on 8 TRN2 cores.

Sharding: 8 cores = 4 batches x 2 roles (one SPMD NEFF, role picked by
partition_id):
  role A (cores 0-3, batch = pid):     queries [0, SPLIT),  kv [0, SPLIT)
  role B (cores 4-7, batch = pid - 4): queries [SPLIT, S),  kv [0, S)
SPLIT=2944 balances causal-attention + projection FLOPs across roles.

Per-core pipeline (bf16 matmuls, fp32 PSUM accumulation):
  1. Project kT/v (then qT) from streamed xT tiles.
  2. Per query block: scores computed transposed (S^T[kv, q]) so the exp
     output P^T feeds the PV matmul directly; causal handled by only
     iterating kv chunks up to the block end plus additive -1e9 masks on
     diagonal chunks; softmax denominator via a ones-vector matmul;
     normalization via K=1 broadcast matmul of the reciprocal fused into
     the PSUM->SBUF copy.
Output per core is O^T [D, S] (its query-column range); host transposes.
"""

import numpy as np
import ml_dtypes

BF16 = ml_dtypes.bfloat16

B, S, D = 4, 4096, 1024
SPLIT = 2816
N_CORES = 8
SCALE = 1.0 / np.sqrt(np.float32(D))  # 1/32
NEG = -1.0e9

_PROGRAM = None


def _role_blocks(q0, q1, m_block):
    """List of (m_start, m_width) query blocks covering [q0, q1)."""
    blocks = []
    m = q0
    while m < q1:
        blocks.append((m, min(m_block, q1 - m)))
        m += m_block
    return blocks


def _build_role(tc, nc, aps, q0, q1, kv_len, m_block, tag, d=D, blocks=None):
    import concourse.bass as bass  # noqa: F401
    from concourse import mybir
    from contextlib import ExitStack

    f32 = mybir.dt.float32
    bf16 = mybir.dt.bfloat16
    Exp = mybir.ActivationFunctionType.Exp
    add_op = mybir.AluOpType.add
    scale = float(1.0 / np.sqrt(np.float32(d)))

    xT, wqT, wkT, wvT, masks, oT = (
        aps["xT"], aps["wqT"], aps["wkT"], aps["wvT"], aps["masks"], aps["oT"],
    )

    DCH = d // 128            # d-chunks
    q_len = q1 - q0
    n_kv = kv_len // 128      # kv chunks
    if blocks is None:
        blocks = _role_blocks(q0, q1, m_block)
    m_block = max(w for _, w in blocks)

    with ExitStack() as ctx:
        # ---- persistent SBUF: kT, v, qT, masks, ones -------------------
        kt_pool = ctx.enter_context(tc.tile_pool(name=f"kt{tag}", bufs=DCH))
        qt_pool = ctx.enter_context(tc.tile_pool(name=f"qt{tag}", bufs=DCH))
        v_pool = ctx.enter_context(tc.tile_pool(name=f"v{tag}", bufs=n_kv))
        misc_pool = ctx.enter_context(tc.tile_pool(name=f"misc{tag}", bufs=1))

        kt = [kt_pool.tile([128, kv_len], bf16, tag="kt", name=f"kt{i}") for i in range(DCH)]
        qt = [qt_pool.tile([128, q_len], bf16, tag="qt", name=f"qt{i}") for i in range(DCH)]
        v = [v_pool.tile([128, d], bf16, tag="v", name=f"v{i}") for i in range(n_kv)]

        masks_sb = misc_pool.tile([128, 4, 512], bf16, tag="masks")
        nc.sync.dma_start(
            masks_sb[:], masks.rearrange("(a p) m -> p a m", p=128)
        )
        ones_row = misc_pool.tile([1, 128], f32, tag="ones_row")
        nc.gpsimd.memset(ones_row[:], 1.0)
        ones_col = misc_pool.tile([128, 1], f32, tag="ones_col")
        nc.gpsimd.memset(ones_col[:], 1.0)

        # ---- phase 1a: kT and v projections ----------------------------
        def s_tiles(lo, hi):
            t = lo
            while t < hi:
                yield (t, min(512, hi - t))
                t += 512

        with tc.tile_pool(name=f"xt{tag}", bufs=12) as xt_pool, \
             tc.tile_pool(name=f"pps{tag}", bufs=4, space="PSUM") as proj_ps:

            def load_xt(s0, sw):
                xts = []
                for j in range(DCH):
                    t = xt_pool.tile([128, 512], bf16, tag="xt", name=f"xt{j}")
                    nc.sync.dma_start(
                        t[:, :sw], xT[j * 128:(j + 1) * 128, s0:s0 + sw]
                    )
                    xts.append(t)
                return xts

            def proj_pass(w_sb, lo, hi, out_cb, first_xts=None):
                # out_cb(i, s0, sw, psum_slice) consumes the [128, sw]
                # projection of d_out chunk i for tokens [s0, s0+sw)
                for s0, sw in s_tiles(lo, hi):
                    xts = first_xts if (first_xts and s0 == lo) else load_xt(s0, sw)
                    for i in range(DCH):
                        ps = proj_ps.tile([128, 512], f32, tag="pps")
                        for j in range(DCH):
                            nc.tensor.matmul(
                                ps[:, :sw],
                                w_sb[:, j, i * 128:(i + 1) * 128],
                                xts[j][:, :sw],
                                start=(j == 0), stop=(j == DCH - 1),
                            )
                        out_cb(i, s0, sw, ps)
                    yield s0, sw, xts

            # per-chunk weight DMAs so the first matmuls start as soon as
            # chunk 0 lands instead of waiting for the whole 2MB transfer
            with tc.tile_pool(name=f"wkv{tag}", bufs=1) as w_pool:
                wk_sb = w_pool.tile([128, DCH, d], bf16, tag="wk")
                wv_sb = w_pool.tile([128, DCH, d], bf16, tag="wv")
                for j in range(DCH):
                    nc.sync.dma_start(wk_sb[:, j, :], wkT[j * 128:(j + 1) * 128, :])
                first_xts = load_xt(0, min(512, kv_len))
                for j in range(DCH):
                    nc.sync.dma_start(wv_sb[:, j, :], wvT[j * 128:(j + 1) * 128, :])

                def kt_cb(i, s0, sw, ps):
                    nc.scalar.copy(kt[i][:, s0:s0 + sw], ps[:, :sw])

                for s0, sw, xts in proj_pass(wk_sb, 0, kv_len, kt_cb,
                                             first_xts=first_xts):
                    # v[s chunk c, d_out] = sum_j (xT[j, c]).T @ WvT[j, :]
                    for c in range(sw // 128):
                        for h0 in range(0, d, 512):
                            hw_ = min(512, d - h0)
                            ps = proj_ps.tile([128, 512], f32, tag="pps")
                            for j in range(DCH):
                                nc.tensor.matmul(
                                    ps[:, :hw_],
                                    xts[j][:, c * 128:(c + 1) * 128],
                                    wv_sb[:, j, h0:h0 + hw_],
                                    start=(j == 0), stop=(j == DCH - 1),
                                )
                            nc.scalar.copy(
                                v[s0 // 128 + c][:, h0:h0 + hw_], ps[:, :hw_]
                            )

            # qT projection (wq pool reuses the freed wk/wv space; per-chunk
            # DMAs keep the WAR stall at the transition ~1 chunk deep)
            with tc.tile_pool(name=f"wq{tag}", bufs=1) as w_pool:
                wq_sb = w_pool.tile([128, DCH, d], bf16, tag="wq")
                for j in range(DCH):
                    nc.sync.dma_start(wq_sb[:, j, :], wqT[j * 128:(j + 1) * 128, :])

                def qt_cb(i, s0, sw, ps):
                    nc.scalar.copy(qt[i][:, s0 - q0:s0 - q0 + sw], ps[:, :sw])

                for _ in proj_pass(wq_sb, q0, q1, qt_cb):
                    pass

        # ---- phase 2: attention per query block ------------------------
        # Diagonal chunks are clipped to their valid column range [lo, mw):
        # for a chunk starting at kv position n0 = m0 + rel (rel >= 0),
        # columns [0, rel) of the block are entirely masked, so QK/exp/PV
        # skip them.
        n_chunks_max = max((m0 + w) // 128 for m0, w in blocks)
        with tc.tile_pool(name=f"pt{tag}", bufs=n_chunks_max + 1) as pt_pool, \
             tc.tile_pool(name=f"att{tag}", bufs=2) as att_sb, \
             tc.tile_pool(name=f"ob{tag}", bufs=3) as out_sb, \
             tc.tile_pool(name=f"st{tag}", bufs=2, space="PSUM") as st_ps, \
             tc.tile_pool(name=f"ot{tag}", bufs=2, space="PSUM") as ot_ps, \
             tc.tile_pool(name=f"bc{tag}", bufs=1, space="PSUM") as bc_ps:
            for m0, mw in blocks:
                mloc = m0 - q0
                n_chunks = (m0 + mw) // 128
                acc = att_sb.tile([128, m_block], f32, tag="acc", name="acc")
                pts = []
                for n in range(n_chunks):
                    rel = n * 128 - m0
                    lo = max(rel, 0)
                    st = st_ps.tile([128, m_block], f32, tag="st")
                    for j in range(DCH):
                        nc.tensor.matmul(
                            st[:, lo:mw],
                            kt[j][:, n * 128:(n + 1) * 128],
                            qt[j][:, mloc + lo:mloc + mw],
                            start=(j == 0), stop=(j == DCH - 1),
                        )
                    if rel >= 0:
                        nc.vector.tensor_tensor(
                            st[:, lo:mw], st[:, lo:mw],
                            masks_sb[:, rel // 128, lo:mw], add_op,
                        )
                    pt = pt_pool.tile([128, m_block], bf16, tag="pt", name="pt")
                    nc.scalar.activation(pt[:, lo:mw], st[:, lo:mw], Exp, scale=scale)
                    pts.append(pt)
                    # accumulate exp tiles (fp32) for the softmax denominator
                    if n == 0:
                        nc.vector.tensor_copy(acc[:, :mw], pt[:, :mw])
                    else:
                        nc.vector.tensor_add(acc[:, lo:mw], acc[:, lo:mw],
                                             pt[:, lo:mw])
                # denominator = partition-sum of acc via one fp32 ones-matmul
                dn_ps = bc_ps.tile([1, m_block], f32, tag="dnp", name="dn_ps")
                nc.tensor.matmul(
                    dn_ps[:, :mw], ones_col[:], acc[:, :mw],
                    start=True, stop=True,
                )
                recip = att_sb.tile([1, m_block], f32, tag="recip")
                nc.vector.reciprocal(recip[:, :mw], dn_ps[:, :mw])
                bcast_ps = bc_ps.tile([128, m_block], f32, tag="bc")
                nc.tensor.matmul(
                    bcast_ps[:, :mw], ones_row[:], recip[:, :mw],
                    start=True, stop=True,
                )
                bcast = att_sb.tile([128, m_block], f32, tag="bcast")
                nc.scalar.copy(bcast[:, :mw], bcast_ps[:, :mw])
                for dd in range(DCH):
                    ot = ot_ps.tile([128, m_block], f32, tag="ot")
                    for n in range(n_chunks):
                        lo = max(n * 128 - m0, 0)
                        nc.tensor.matmul(
                            ot[:, lo:mw],
                            v[n][:, dd * 128:(dd + 1) * 128],
                            pts[n][:, lo:mw],
                            start=(n == 0), stop=(n == n_chunks - 1),
                        )
                    o = out_sb.tile([128, m_block], f32, tag="o")
                    nc.vector.tensor_mul(o[:, :mw], ot[:, :mw], bcast[:, :mw])
                    nc.sync.dma_start(
                        oT[dd * 128:(dd + 1) * 128, m0:m0 + mw], o[:, :mw]
                    )


def build_program(s=S, d=D, split=SPLIT, m_block_a=512, m_block_b=384,
                  n_cores=N_CORES):
    """Build and compile the SPMD Bass program. Returns the Bacc object."""
    import concourse.tile as tile
    from concourse import bacc, mybir

    nc = bacc.Bacc(
        "TRN2",
        target_bir_lowering=False,
        debug=False,
        enable_asserts=False,
        num_devices=n_cores,
    )
    bf16 = mybir.dt.bfloat16
    f32 = mybir.dt.float32
    aps = {
        "xT": nc.dram_tensor("xT", [d, s], bf16, kind="ExternalInput").ap(),
        "wqT": nc.dram_tensor("wqT", [d, d], bf16, kind="ExternalInput").ap(),
        "wkT": nc.dram_tensor("wkT", [d, d], bf16, kind="ExternalInput").ap(),
        "wvT": nc.dram_tensor("wvT", [d, d], bf16, kind="ExternalInput").ap(),
        "masks": nc.dram_tensor("masks", [512, 512], bf16, kind="ExternalInput").ap(),
        "oT": nc.dram_tensor("oT", [d, s], f32, kind="ExternalOutput").ap(),
    }
    with tile.TileContext(nc) as tc:
        pid = nc.partition_id()
        with tc.If(pid < n_cores // 2) as cmp:
            _build_role(tc, nc, aps, 0, split, split, m_block_a, "a", d=d)
        with cmp.Else():
            if (s, split) == (4096, 2816):
                blocks_b = [(2816, 512), (3328, 384), (3712, 384)]
            else:
                blocks_b = None
            _build_role(tc, nc, aps, split, s, s, m_block_b, "b", d=d,
                        blocks=blocks_b)
    nc.compile()
    return nc


def host_masks():
    part = np.arange(128, dtype=np.int64)[:, None]
    col = np.arange(512, dtype=np.int64)[None, :]
    m = np.zeros((4, 128, 512), np.float32)
    for r in range(4):
        m[r] = np.where(col >= part + r * 128, 0.0, NEG)
    return np.ascontiguousarray(m.reshape(512, 512).astype(BF16))


def make_in_maps(x, Wq, Wk, Wv):
    wqT = np.ascontiguousarray(Wq.T.astype(BF16))
    wkT = np.ascontiguousarray(Wk.T.astype(BF16))
    wvT = np.ascontiguousarray(Wv.T.astype(BF16))
    masks = host_masks()
    xT = np.ascontiguousarray(x.astype(BF16).transpose(0, 2, 1))  # [B, D, S]
    in_maps = []
    for c in range(N_CORES):
        b = c % B
        in_maps.append({
            "xT": xT[b], "wqT": wqT, "wkT": wkT, "wvT": wvT, "masks": masks,
        })
    return in_maps


def gather_output(results):
    out = np.empty((B, S, D), np.float32)
    for b in range(B):
        oA = results[b]["oT"]        # [D, S], valid cols [0, SPLIT)
        oB = results[B + b]["oT"]    # [D, S], valid cols [SPLIT, S)
        out[b, :SPLIT] = oA[:, :SPLIT].T
        out[b, SPLIT:] = oB[:, SPLIT:].T
    return out


def get_program():
    global _PROGRAM
    if _PROGRAM is None:
        _PROGRAM = build_program()
    return _PROGRAM


def kernel(x, Wq, Wk, Wv, _trace=False, _trace_cores=None):
    from concourse import bass_utils

    nc = get_program()
    in_maps = make_in_maps(x, Wq, Wk, Wv)
    res = bass_utils.run_bass_kernel_spmd(
        nc, in_maps, core_ids=list(range(N_CORES)),
        trace=_trace, trace_cores=_trace_cores,
    )
    out = gather_output(res.results)
    if _trace:
        kernel.last_results = res
    return out


# revision 12
# speedup vs baseline: 1.1234x; 1.0257x over previous
"""Causal single-head attention (B=4, S=4096, D=1024, fp32) on 8 TRN2 cores.

Sharding: 8 cores = 4 batches x 2 roles (one SPMD NEFF, role picked by
partition_id):
  role A (cores 0-3, batch = pid):     queries [0, SPLIT),  kv [0, SPLIT)
  role B (cores 4-7, batch = pid - 4): queries [SPLIT, S),  kv [0, S)
SPLIT=2944 balances causal-attention + projection FLOPs across roles.

Per-core pipeline (bf16 matmuls, fp32 PSUM accumulation):
  1. Project kT/v (then qT) from streamed xT tiles.
  2. Per query block: scores computed transposed (S^T[kv, q]) so the exp
     output P^T feeds the PV matmul directly; causal handled by only
     iterating kv chunks up to the block end plus additive -1e9 masks on
     diagonal chunks; softmax denominator via a ones-vector matmul;
     normalization via K=1 broadcast matmul of the reciprocal fused into
     the PSUM->SBUF copy.
Output per core is O^T [D, S] (its query-column range); host transposes.
"""

import numpy as np
import ml_dtypes

BF16 = ml_dtypes.bfloat16

B, S, D = 4, 4096, 1024
SPLIT = 2816
N_CORES = 8
SCALE = 1.0 / np.sqrt(np.float32(D))  # 1/32
NEG = -1.0e9

_PROGRAM = None


def _role_blocks(q0, q1, m_block):
    """List of (m_start, m_width) query blocks covering [q0, q1)."""
    blocks = []
    m = q0
    while m < q1:
        blocks.append((m, min(m_block, q1 - m)))
        m += m_block
    return blocks


def _build_role(tc, nc, aps, q0, q1, kv_len, m_block, tag, d=D, blocks=None):
    import concourse.bass as bass  # noqa: F401
    from concourse import mybir
    from contextlib import ExitStack

    f32 = mybir.dt.float32
    bf16 = mybir.dt.bfloat16
    Exp = mybir.ActivationFunctionType.Exp
    add_op = mybir.AluOpType.add
    scale = float(1.0 / np.sqrt(np.float32(d)))

    xT, wqT, wkT, wvT, masks, oT = (
        aps["xT"], aps["wqT"], aps["wkT"], aps["wvT"], aps["masks"], aps["oT"],
    )

    DCH = d // 128            # d-chunks
    q_len = q1 - q0
    n_kv = kv_len // 128      # kv chunks
    if blocks is None:
        blocks = _role_blocks(q0, q1, m_block)
    m_block = max(w for _, w in blocks)

    with ExitStack() as ctx:
        # ---- persistent SBUF: kT, v, qT, masks, ones -------------------
        kt_pool = ctx.enter_context(tc.tile_pool(name=f"kt{tag}", bufs=DCH))
        qt_pool = ctx.enter_context(tc.tile_pool(name=f"qt{tag}", bufs=DCH))
        v_pool = ctx.enter_context(tc.tile_pool(name=f"v{tag}", bufs=n_kv))
        misc_pool = ctx.enter_context(tc.tile_pool(name=f"misc{tag}", bufs=1))

        kt = [kt_pool.tile([128, kv_len], bf16, tag="kt", name=f"kt{i}") for i in range(DCH)]
        qt = [qt_pool.tile([128, q_len], bf16, tag="qt", name=f"qt{i}") for i in range(DCH)]
        v = [v_pool.tile([128, d], bf16, tag="v", name=f"v{i}") for i in range(n_kv)]

        masks_sb = misc_pool.tile([128, 4, 512], bf16, tag="masks")
        nc.sync.dma_start(
            masks_sb[:], masks.rearrange("(a p) m -> p a m", p=128)
        )
        ones_row = misc_pool.tile([1, 128], f32, tag="ones_row")
        nc.gpsimd.memset(ones_row[:], 1.0)
        ones_col = misc_pool.tile([128, 1], f32, tag="ones_col")
        nc.gpsimd.memset(ones_col[:], 1.0)

        # ---- phase 1a: kT and v projections ----------------------------
        def s_tiles(lo, hi):
            t = lo
            while t < hi:
                yield (t, min(512, hi - t))
                t += 512

        with tc.tile_pool(name=f"xt{tag}", bufs=12) as xt_pool, \
             tc.tile_pool(name=f"pps{tag}", bufs=4, space="PSUM") as proj_ps:

            def load_xt(s0, sw):
                xts = []
                for j in range(DCH):
                    t = xt_pool.tile([128, 512], bf16, tag="xt", name=f"xt{j}")
                    nc.sync.dma_start(
                        t[:, :sw], xT[j * 128:(j + 1) * 128, s0:s0 + sw]
                    )
                    xts.append(t)
                return xts

            def proj_pass(w_sb, lo, hi, out_cb, first_xts=None):
                # out_cb(i, s0, sw, psum_slice) consumes the [128, sw]
                # projection of d_out chunk i for tokens [s0, s0+sw)
                for s0, sw in s_tiles(lo, hi):
                    xts = first_xts if (first_xts and s0 == lo) else load_xt(s0, sw)
                    for i in range(DCH):
                        ps = proj_ps.tile([128, 512], f32, tag="pps")
                        for j in range(DCH):
                            nc.tensor.matmul(
                                ps[:, :sw],
                                w_sb[:, j, i * 128:(i + 1) * 128],
                                xts[j][:, :sw],
                                start=(j == 0), stop=(j == DCH - 1),
                            )
                        out_cb(i, s0, sw, ps)
                    yield s0, sw, xts

            # per-chunk weight DMAs so the first matmuls start as soon as
            # chunk 0 lands instead of waiting for the whole 2MB transfer
            with tc.tile_pool(name=f"wkv{tag}", bufs=1) as w_pool:
                wk_sb = w_pool.tile([128, DCH, d], bf16, tag="wk")
                wv_sb = w_pool.tile([128, DCH, d], bf16, tag="wv")
                sw0 = min(512, kv_len)
                first_xts = []
                for j in range(DCH):
                    nc.sync.dma_start(wk_sb[:, j, :], wkT[j * 128:(j + 1) * 128, :])
                    t = xt_pool.tile([128, 512], bf16, tag="xt", name=f"xtf{j}")
                    nc.sync.dma_start(t[:, :sw0], xT[j * 128:(j + 1) * 128, 0:sw0])
                    first_xts.append(t)
                for j in range(DCH):
                    nc.sync.dma_start(wv_sb[:, j, :], wvT[j * 128:(j + 1) * 128, :])

                def kt_cb(i, s0, sw, ps):
                    nc.scalar.copy(kt[i][:, s0:s0 + sw], ps[:, :sw])

                for s0, sw, xts in proj_pass(wk_sb, 0, kv_len, kt_cb,
                                             first_xts=first_xts):
                    # v[s chunk c, d_out] = sum_j (xT[j, c]).T @ WvT[j, :]
                    for c in range(sw // 128):
                        for h0 in range(0, d, 512):
                            hw_ = min(512, d - h0)
                            ps = proj_ps.tile([128, 512], f32, tag="pps")
                            for j in range(DCH):
                                nc.tensor.matmul(
                                    ps[:, :hw_],
                                    xts[j][:, c * 128:(c + 1) * 128],
                                    wv_sb[:, j, h0:h0 + hw_],
                                    start=(j == 0), stop=(j == DCH - 1),
                                )
                            nc.scalar.copy(
                                v[s0 // 128 + c][:, h0:h0 + hw_], ps[:, :hw_]
                            )

            # qT projection (wq pool reuses the freed wk/wv space; per-chunk
            # DMAs keep the WAR stall at the transition ~1 chunk deep)
            with tc.tile_pool(name=f"wq{tag}", bufs=1) as w_pool:
                wq_sb = w_pool.tile([128, DCH, d], bf16, tag="wq")
                for j in range(DCH):
                    nc.sync.dma_start(wq_sb[:, j, :], wqT[j * 128:(j + 1) * 128, :])

                def qt_cb(i, s0, sw, ps):
                    nc.scalar.copy(qt[i][:, s0 - q0:s0 - q0 + sw], ps[:, :sw])

                for _ in proj_pass(wq_sb, q0, q1, qt_cb):
                    pass

        # ---- phase 2: attention per query block ------------------------
        # Diagonal chunks are clipped to their valid column range [lo, mw):
        # for a chunk starting at kv position n0 = m0 + rel (rel >= 0),
        # columns [0, rel) of the block are entirely masked, so QK/exp/PV
        # skip them.
        n_chunks_max = max((m0 + w) // 128 for m0, w in blocks)
        with tc.tile_pool(name=f"pt{tag}", bufs=n_chunks_max + 4) as pt_pool, \
             tc.tile_pool(name=f"att{tag}", bufs=2) as att_sb, \
             tc.tile_pool(name=f"ob{tag}", bufs=3) as out_sb, \
             tc.tile_pool(name=f"st{tag}", bufs=3, space="PSUM") as st_ps, \
             tc.tile_pool(name=f"ot{tag}", bufs=3, space="PSUM") as ot_ps, \
             tc.tile_pool(name=f"bc{tag}", bufs=1, space="PSUM") as bc_ps:
            for m0, mw in blocks:
                mloc = m0 - q0
                n_chunks = (m0 + mw) // 128
                acc = att_sb.tile([128, m_block], f32, tag="acc", name="acc")
                pts = []
                for n in range(n_chunks):
                    rel = n * 128 - m0
                    lo = max(rel, 0)
                    st = st_ps.tile([128, m_block], f32, tag="st")
                    for j in range(DCH):
                        nc.tensor.matmul(
                            st[:, lo:mw],
                            kt[j][:, n * 128:(n + 1) * 128],
                            qt[j][:, mloc + lo:mloc + mw],
                            start=(j == 0), stop=(j == DCH - 1),
                        )
                    if rel >= 0:
                        nc.vector.tensor_tensor(
                            st[:, lo:mw], st[:, lo:mw],
                            masks_sb[:, rel // 128, lo:mw], add_op,
                        )
                    pt = pt_pool.tile([128, m_block], bf16, tag="pt", name="pt")
                    nc.scalar.activation(pt[:, lo:mw], st[:, lo:mw], Exp, scale=scale)
                    pts.append(pt)
                    # accumulate exp tiles (fp32) for the softmax denominator
                    if n == 0:
                        nc.vector.tensor_copy(acc[:, :mw], pt[:, :mw])
                    else:
                        nc.vector.tensor_add(acc[:, lo:mw], acc[:, lo:mw],
                                             pt[:, lo:mw])
                # denominator = partition-sum of acc via one fp32 ones-matmul
                dn_ps = bc_ps.tile([1, m_block], f32, tag="dnp", name="dn_ps")
                nc.tensor.matmul(
                    dn_ps[:, :mw], ones_col[:], acc[:, :mw],
                    start=True, stop=True,
                )
                recip = att_sb.tile([1, m_block], f32, tag="recip")
                nc.vector.reciprocal(recip[:, :mw], dn_ps[:, :mw])
                bcast_ps = bc_ps.tile([128, m_block], f32, tag="bc")
                nc.tensor.matmul(
                    bcast_ps[:, :mw], ones_row[:], recip[:, :mw],
                    start=True, stop=True,
                )
                bcast = att_sb.tile([128, m_block], f32, tag="bcast")
                nc.scalar.copy(bcast[:, :mw], bcast_ps[:, :mw])
                for dd in range(DCH):
                    ot = ot_ps.tile([128, m_block], f32, tag="ot")
                    for n in range(n_chunks):
                        lo = max(n * 128 - m0, 0)
                        nc.tensor.matmul(
                            ot[:, lo:mw],
                            v[n][:, dd * 128:(dd + 1) * 128],
                            pts[n][:, lo:mw],
                            start=(n == 0), stop=(n == n_chunks - 1),
                        )
                    o = out_sb.tile([128, m_block], f32, tag="o")
                    nc.vector.tensor_mul(o[:, :mw], ot[:, :mw], bcast[:, :mw])
                    nc.sync.dma_start(
                        oT[dd * 128:(dd + 1) * 128, m0:m0 + mw], o[:, :mw]
                    )


def build_program(s=S, d=D, split=SPLIT, m_block_a=512, m_block_b=384,
                  n_cores=N_CORES):
    """Build and compile the SPMD Bass program. Returns the Bacc object."""
    import concourse.tile as tile
    from concourse import bacc, mybir

    nc = bacc.Bacc(
        "TRN2",
        target_bir_lowering=False,
        debug=False,
        enable_asserts=False,
        num_devices=n_cores,
    )
    bf16 = mybir.dt.bfloat16
    f32 = mybir.dt.float32
    aps = {
        "xT": nc.dram_tensor("xT", [d, s], bf16, kind="ExternalInput").ap(),
        "wqT": nc.dram_tensor("wqT", [d, d], bf16, kind="ExternalInput").ap(),
        "wkT": nc.dram_tensor("wkT", [d, d], bf16, kind="ExternalInput").ap(),
        "wvT": nc.dram_tensor("wvT", [d, d], bf16, kind="ExternalInput").ap(),
        "masks": nc.dram_tensor("masks", [512, 512], bf16, kind="ExternalInput").ap(),
        "oT": nc.dram_tensor("oT", [d, s], f32, kind="ExternalOutput").ap(),
    }
    with tile.TileContext(nc) as tc:
        pid = nc.partition_id()
        with tc.If(pid < n_cores // 2) as cmp:
            _build_role(tc, nc, aps, 0, split, split, m_block_a, "a", d=d)
        with cmp.Else():
            if (s, split) == (4096, 2816):
                blocks_b = [(2816, 512), (3328, 384), (3712, 384)]
            else:
                blocks_b = None
            _build_role(tc, nc, aps, split, s, s, m_block_b, "b", d=d,
                        blocks=blocks_b)
    nc.compile()
    return nc


def host_masks():
    part = np.arange(128, dtype=np.int64)[:, None]
    col = np.arange(512, dtype=np.int64)[None, :]
    m = np.zeros((4, 128, 512), np.float32)
    for r in range(4):
        m[r] = np.where(col >= part + r * 128, 0.0, NEG)
    return np.ascontiguousarray(m.reshape(512, 512).astype(BF16))


def make_in_maps(x, Wq, Wk, Wv):
    wqT = np.ascontiguousarray(Wq.T.astype(BF16))
    wkT = np.ascontiguousarray(Wk.T.astype(BF16))
    wvT = np.ascontiguousarray(Wv.T.astype(BF16))
    masks = host_masks()
    xT = np.ascontiguousarray(x.astype(BF16).transpose(0, 2, 1))  # [B, D, S]
    in_maps = []
    for c in range(N_CORES):
        b = c % B
        in_maps.append({
            "xT": xT[b], "wqT": wqT, "wkT": wkT, "wvT": wvT, "masks": masks,
        })
    return in_maps


def gather_output(results):
    out = np.empty((B, S, D), np.float32)
    for b in range(B):
        oA = results[b]["oT"]        # [D, S], valid cols [0, SPLIT)
        oB = results[B + b]["oT"]    # [D, S], valid cols [SPLIT, S)
        out[b, :SPLIT] = oA[:, :SPLIT].T
        out[b, SPLIT:] = oB[:, SPLIT:].T
    return out


def get_program():
    global _PROGRAM
    if _PROGRAM is None:
        _PROGRAM = build_program()
    return _PROGRAM


def kernel(x, Wq, Wk, Wv, _trace=False, _trace_cores=None):
    from concourse import bass_utils

    nc = get_program()
    in_maps = make_in_maps(x, Wq, Wk, Wv)
    res = bass_utils.run_bass_kernel_spmd(
        nc, in_maps, core_ids=list(range(N_CORES)),
        trace=_trace, trace_cores=_trace_cores,
    )
    out = gather_output(res.results)
    if _trace:
        kernel.last_results = res
    return out
